# revision 5
# baseline (speedup 1.0000x reference)
"""AlexNet forward (batch 128) on 8 TRN2 NeuronCores.

Strategy:
- Data-parallel conv stack: 16 images/core. Convs as shift-matmuls in bf16
  (fp32 PSUM accum). conv1's 11x11/s4 conv is rewritten via space-to-depth
  (4x4 phases) into a 3x3/s1 conv over 48 channels.
- LRN via band-matrix matmul for the channel-window sum of squares, then a
  first-order expansion (k + a*S)^-b ~= C0 - C1*S (exact to ~1e-8 here since
  a*S << k).
- Maxpool 3x3/s2 as strided DVE max ops.
- FC layers tensor-sharded 8 ways (512/512/125 output columns per core) over
  the full batch-128, with AllGather collectives between layers.
Host side: pad/space-to-depth/transpose/cast of inputs+weights, final
concat+transpose of the per-core FC8 slices.
"""
import numpy as np
import ml_dtypes

import concourse.bass as bass
import concourse.mybir as mybir
import concourse.tile as tile
from concourse import bacc
from concourse.bass_utils import run_bass_kernel_spmd

F32 = mybir.dt.float32
BF16 = mybir.dt.float16
BF = np.float16

N_CORES = 8
C0 = float(2.0 ** -0.75)
C1 = float(0.75 * 1e-4 * 2.0 ** -1.75)
RELU = mybir.ActivationFunctionType.Relu
COPY = mybir.ActivationFunctionType.Copy
MAX = mybir.AluOpType.max
MULT = mybir.AluOpType.mult


def _chunks(total, maxn):
    n = -(-total // maxn)
    base, rem = divmod(total, n)
    out, s = [], 0
    for i in range(n):
        ln = base + (1 if i < rem else 0)
        out.append((s, ln))
        s += ln
    return out


def _groups(n, g):
    out, s = [], 0
    while s < n:
        out.append((s, min(g, n - s)))
        s += min(g, n - s)
    return out


def _pool_cols(v, wo):
    """3-wide stride-2 max along the last dim of a [P, H, Wi] view -> [P, H, wo]."""
    return [v[:, :, s: s + 2 * (wo - 1) + 1: 2] for s in range(3)]


def _pool_rows(v, ho):
    return [v[:, s: s + 2 * (ho - 1) + 1: 2, :] for s in range(3)]


def build(n_img=16):
    NB = N_CORES * n_img
    nc = bacc.Bacc(num_devices=N_CORES)

    def inp(name, shape, dt=BF16):
        return nc.declare_dram_parameter(name, list(shape), dt, isOutput=False)

    x = inp("x", (n_img, 48, 57, 57))
    w1 = inp("w1", (9, 48, 96))
    w2 = inp("w2", (25, 96, 256))
    w3 = inp("w3", (9, 256, 384))
    w4 = inp("w4", (9, 384, 384))
    w5 = inp("w5", (9, 384, 256))
    w6t = inp("w6t", (72, 128, 512))
    w7t = inp("w7t", (32, 128, 512))
    w8t = inp("w8t", (32, 128, 125))
    b1s = inp("b1s", (96, 1), F32)
    b2s = inp("b2s", (128, 2), F32)
    b3s = inp("b3s", (128, 3), F32)
    b4s = inp("b4s", (128, 3), F32)
    b5s = inp("b5s", (128, 2), F32)
    b6s = inp("b6s", (128, 4), F32)
    b7s = inp("b7s", (128, 4), F32)
    b8s = inp("b8s", (125, 1), F32)
    band96d = inp("band96", (96, 96))
    band256d = inp("band256", (128, 2, 2, 128))

    out8 = nc.declare_dram_parameter("out8", [125, NB], F32, isOutput=True)

    h5_loc = nc.dram_tensor("h5_loc", [n_img, 2, 128, 36], BF16)
    h5_all = nc.dram_tensor("h5_all", [NB, 9216], BF16, addr_space="Shared")
    h6_loc = nc.dram_tensor("h6_loc", [512, NB], BF16)
    h6_all = nc.dram_tensor("h6_all", [4096, NB], BF16, addr_space="Shared")
    h7_loc = nc.dram_tensor("h7_loc", [512, NB], BF16)
    h7_all = nc.dram_tensor("h7_all", [4096, NB], BF16, addr_space="Shared")

    RG = [list(range(N_CORES))]

    with tile.TileContext(nc, num_cores=N_CORES) as tc:
        with (
            tc.tile_pool(name="consts", bufs=1) as consts,
            tc.tile_pool(name="convw", bufs=1) as convw,
            tc.tile_pool(name="xin", bufs=2) as xinp,
            tc.tile_pool(name="act", bufs=2) as actp,
            tc.tile_pool(name="chunk", bufs=2) as chp,
            tc.tile_pool(name="grp", bufs=2) as grpp,
            tc.tile_pool(name="fcw", bufs=8) as fcwp,
            tc.tile_pool(name="fca", bufs=4) as fcap,
            tc.tile_pool(name="fco", bufs=1) as fcop,
            tc.tile_pool(name="convps", bufs=2, space="PSUM") as convps,
            tc.tile_pool(name="lrnps", bufs=2, space="PSUM") as lrnps,
            tc.tile_pool(name="fcps", bufs=4, space="PSUM") as fcps,
        ):
            # ---------------- constants / conv weights ----------------
            b1t = consts.tile([96, 1], F32); nc.sync.dma_start(out=b1t, in_=b1s[:, :])
            b2t = consts.tile([128, 2], F32); nc.sync.dma_start(out=b2t, in_=b2s[:, :])
            b3t = consts.tile([128, 3], F32); nc.sync.dma_start(out=b3t, in_=b3s[:, :])
            b4t = consts.tile([128, 3], F32); nc.sync.dma_start(out=b4t, in_=b4s[:, :])
            b5t = consts.tile([128, 2], F32); nc.sync.dma_start(out=b5t, in_=b5s[:, :])
            b6t = consts.tile([128, 4], F32); nc.sync.dma_start(out=b6t, in_=b6s[:, :])
            b7t = consts.tile([128, 4], F32); nc.sync.dma_start(out=b7t, in_=b7s[:, :])
            b8t = consts.tile([125, 1], F32); nc.sync.dma_start(out=b8t, in_=b8s[:, :])
            band96 = consts.tile([96, 96], BF16)
            nc.sync.dma_start(out=band96, in_=band96d[:, :])
            band256 = consts.tile([128, 2, 2, 128], BF16)
            nc.sync.dma_start(out=band256, in_=band256d[:, :, :, :])

            w1t = convw.tile([48, 9, 96], BF16)
            nc.sync.dma_start(out=w1t, in_=w1.ap().rearrange("p c o -> c p o"))
            w2t = convw.tile([96, 25, 256], BF16)
            nc.sync.dma_start(out=w2t, in_=w2.ap().rearrange("p c o -> c p o"))
            w3t = convw.tile([128, 2, 9, 384], BF16, tag="w3t")
            w4t = convw.tile([128, 3, 9, 384], BF16, tag="w4t")
            w5t = convw.tile([128, 3, 9, 256], BF16, tag="w5t")
            for kt in range(2):
                nc.sync.dma_start(
                    out=w3t[:, kt, :, :],
                    in_=w3[:, kt * 128:(kt + 1) * 128, :].rearrange("p c o -> c p o"))
            for kt in range(3):
                nc.sync.dma_start(
                    out=w4t[:, kt, :, :],
                    in_=w4[:, kt * 128:(kt + 1) * 128, :].rearrange("p c o -> c p o"))
            for kt in range(3):
                nc.sync.dma_start(
                    out=w5t[:, kt, :, :],
                    in_=w5[:, kt * 128:(kt + 1) * 128, :].rearrange("p c o -> c p o"))

            CH55 = _chunks(55, 9)   # conv1/lrn1 row chunks (N<=495)
            CH27 = _chunks(27, 18)  # conv2 row chunks (N<=486)

            # ---------------- conv stack, grouped by 3 images ----------------
            for (g0, glen) in _groups(n_img, 3):
                in3 = grpp.tile([128, 2, 3, 15, 15], BF16, tag="in3")
                nc.gpsimd.memset(in3[:, :, :, :, :], 0.0)
                for i in range(g0, g0 + glen):
                    sl = i - g0
                    # ---- conv1: [48,57,57] -> relu -> r1 [96,55,55]
                    xin = xinp.tile([48, 57, 57], BF16, tag="xin")
                    nc.sync.dma_start(out=xin, in_=x[i])
                    r1 = actp.tile([96, 55, 55], BF16, tag="r1")
                    for (r0, nr) in CH55:
                        ps = convps.tile([96, nr, 55], F32, tag="cps")
                        k = 0
                        for dy in range(3):
                            for dx in range(3):
                                nc.tensor.matmul(
                                    ps,
                                    w1t[:, dy * 3 + dx, :],
                                    xin[:, dy + r0: dy + r0 + nr, dx: dx + 55],
                                    start=(k == 0), stop=(k == 8))
                                k += 1
                        nc.scalar.activation(r1[:, r0:r0 + nr, :], ps, RELU, bias=b1t[:, 0:1])
                    # ---- lrn1, in place: r1 <- r1 * (C0 - C1 * band96 @ r1^2)
                    for (r0, nr) in CH55:
                        rc = r1[:, r0:r0 + nr, :]
                        t1 = chp.tile([96, nr, 55], BF16, tag="t1")
                        nc.vector.tensor_mul(t1, rc, rc)
                        sp = lrnps.tile([96, nr, 55], F32, tag="lps")
                        nc.tensor.matmul(sp, band96, t1, start=True, stop=True)
                        wc = chp.tile([96, nr, 55], F32, tag="wc")
                        nc.scalar.activation(wc, sp, COPY, bias=C0, scale=-C1)
                        nc.vector.tensor_tensor(out=rc, in0=rc, in1=wc, op=MULT)
                    # ---- pool1 -> in2 [96,31,31] (border zero)
                    in2 = actp.tile([96, 31, 31], BF16, tag="in2")
                    nc.gpsimd.memset(in2[:, :, :], 0.0)
                    tmp1 = chp.tile([96, 55, 27], BF16, tag="tmp1")
                    ca, cb, cc = _pool_cols(r1, 27)
                    nc.vector.tensor_tensor(out=tmp1, in0=ca, in1=cb, op=MAX)
                    nc.vector.tensor_tensor(out=tmp1, in0=tmp1, in1=cc, op=MAX)
                    ra, rb, rw = _pool_rows(tmp1, 27)
                    p1v = in2[:, 2:29, 2:29]
                    nc.vector.tensor_tensor(out=p1v, in0=ra, in1=rb, op=MAX)
                    nc.vector.tensor_tensor(out=p1v, in0=p1v, in1=rw, op=MAX)
                    # ---- conv2 -> r2 [128,2,27,27]
                    r2 = actp.tile([128, 2, 27, 27], BF16, tag="r2")
                    for (r0, nr) in CH27:
                        for mt in range(2):
                            ps = convps.tile([128, nr, 27], F32, tag="cps")
                            k = 0
                            for ky in range(5):
                                for kx in range(5):
                                    nc.tensor.matmul(
                                        ps,
                                        w2t[:, ky * 5 + kx, mt * 128:(mt + 1) * 128],
                                        in2[:, ky + r0: ky + r0 + nr, kx: kx + 27],
                                        start=(k == 0), stop=(k == 24))
                                    k += 1
                            nc.scalar.activation(r2[:, mt, r0:r0 + nr, :], ps, RELU,
                                                 bias=b2t[:, mt:mt + 1])
                    # ---- lrn2 in place on r2
                    for (r0, nr) in CH27:
                        t2 = chp.tile([128, 2, nr, 27], BF16, tag="t2")
                        for kt in range(2):
                            rc = r2[:, kt, r0:r0 + nr, :]
                            nc.vector.tensor_mul(t2[:, kt, :, :], rc, rc)
                        for mt in range(2):
                            sp = lrnps.tile([128, nr, 27], F32, tag="lps")
                            for kt in range(2):
                                nc.tensor.matmul(sp, band256[:, kt, mt, :], t2[:, kt, :, :],
                                                 start=(kt == 0), stop=(kt == 1))
                            wc = chp.tile([128, nr, 27], F32, tag="wc2")
                            nc.scalar.activation(wc, sp, COPY, bias=C0, scale=-C1)
                            rc = r2[:, mt, r0:r0 + nr, :]
                            nc.vector.tensor_tensor(out=rc, in0=rc, in1=wc, op=MULT)
                    # ---- pool2 -> in3[:, kt, sl, 1:14, 1:14]
                    for kt in range(2):
                        tmp2 = chp.tile([128, 27, 13], BF16, tag="tmp2")
                        ca, cb, cc = _pool_cols(r2[:, kt, :, :], 13)
                        nc.vector.tensor_tensor(out=tmp2, in0=ca, in1=cb, op=MAX)
                        nc.vector.tensor_tensor(out=tmp2, in0=tmp2, in1=cc, op=MAX)
                        ra, rb, rw = _pool_rows(tmp2, 13)
                        pv = in3[:, kt, sl, 1:14, 1:14]
                        nc.vector.tensor_tensor(out=pv, in0=ra, in1=rb, op=MAX)
                        nc.vector.tensor_tensor(out=pv, in0=pv, in1=rw, op=MAX)

                # ---- conv3 (group) -> in4 [128,3,3,15,15]
                in4 = grpp.tile([128, 3, 3, 15, 15], BF16, tag="in4")
                nc.gpsimd.memset(in4[:, :, :, :, :], 0.0)
                for mt in range(3):
                    ps = convps.tile([128, glen, 13, 13], F32, tag="cps")
                    k, klast = 0, 2 * 9 - 1
                    for kt in range(2):
                        for dy in range(3):
                            for dx in range(3):
                                nc.tensor.matmul(
                                    ps,
                                    w3t[:, kt, dy * 3 + dx, mt * 128:(mt + 1) * 128],
                                    in3[:, kt, 0:glen, dy:dy + 13, dx:dx + 13],
                                    start=(k == 0), stop=(k == klast))
                                k += 1
                    nc.scalar.activation(in4[:, mt, 0:glen, 1:14, 1:14], ps, RELU,
                                         bias=b3t[:, mt:mt + 1])
                # ---- conv4 (group) -> in5
                in5 = grpp.tile([128, 3, 3, 15, 15], BF16, tag="in5")
                nc.gpsimd.memset(in5[:, :, :, :, :], 0.0)
                for mt in range(3):
                    ps = convps.tile([128, glen, 13, 13], F32, tag="cps")
                    k, klast = 0, 3 * 9 - 1
                    for kt in range(3):
                        for dy in range(3):
                            for dx in range(3):
                                nc.tensor.matmul(
                                    ps,
                                    w4t[:, kt, dy * 3 + dx, mt * 128:(mt + 1) * 128],
                                    in4[:, kt, 0:glen, dy:dy + 13, dx:dx + 13],
                                    start=(k == 0), stop=(k == klast))
                                k += 1
                    nc.scalar.activation(in5[:, mt, 0:glen, 1:14, 1:14], ps, RELU,
                                         bias=b4t[:, mt:mt + 1])
                # ---- conv5 (group) -> r5 [128,2,g,13,13]
                r5 = grpp.tile([128, 2, 3, 13, 13], BF16, tag="r5")
                for mt in range(2):
                    ps = convps.tile([128, glen, 13, 13], F32, tag="cps")
                    k, klast = 0, 3 * 9 - 1
                    for kt in range(3):
                        for dy in range(3):
                            for dx in range(3):
                                nc.tensor.matmul(
                                    ps,
                                    w5t[:, kt, dy * 3 + dx, mt * 128:(mt + 1) * 128],
                                    in5[:, kt, 0:glen, dy:dy + 13, dx:dx + 13],
                                    start=(k == 0), stop=(k == klast))
                                k += 1
                    nc.scalar.activation(r5[:, mt, 0:glen, :, :], ps, RELU,
                                         bias=b5t[:, mt:mt + 1])
                # ---- pool5 -> h5_loc
                for i in range(g0, g0 + glen):
                    sl = i - g0
                    for kt in range(2):
                        tmp5 = chp.tile([128, 13, 6], BF16, tag="tmp5")
                        ca, cb, cc = _pool_cols(r5[:, kt, sl, :, :], 6)
                        nc.vector.tensor_tensor(out=tmp5, in0=ca, in1=cb, op=MAX)
                        nc.vector.tensor_tensor(out=tmp5, in0=tmp5, in1=cc, op=MAX)
                        h5sb = chp.tile([128, 6, 6], BF16, tag="h5sb")
                        ra, rb, rw = _pool_rows(tmp5, 6)
                        nc.vector.tensor_tensor(out=h5sb, in0=ra, in1=rb, op=MAX)
                        nc.vector.tensor_tensor(out=h5sb, in0=h5sb, in1=rw, op=MAX)
                        nc.sync.dma_start(out=h5_loc[i, kt], in_=h5sb)

            # ---------------- FC phase ----------------
            nc.gpsimd.collective_compute(
                "AllGather", mybir.AluOpType.bypass, replica_groups=RG,
                ins=[h5_loc.ap().opt()], outs=[h5_all.ap().opt()])

            # fc6: h6[o, b] for o in this core's 512-slice
            psum6 = []
            for _i in range(4):
                p6 = fcps.tile([128, NB], F32, tag="fcps")
                psum6.append(p6)
            for kt in range(72):
                hk = fcap.tile([128, NB], BF16, tag="h5k")
                src = h5_all[:, kt * 128:(kt + 1) * 128]
                if NB % 16 == 0:
                    nc.sync.dma_start(out=hk, in_=src, transpose=True)
                else:
                    nc.sync.dma_start(out=hk, in_=src.rearrange("a b -> b a"))
                wt = fcwp.tile([128, 512], BF16, tag="w6")
                nc.sync.dma_start(out=wt, in_=w6t[kt])
                for mt in range(4):
                    nc.tensor.matmul(psum6[mt], wt[:, mt * 128:(mt + 1) * 128],
                                     hk, start=(kt == 0), stop=(kt == 71))
            h6sb = fcop.tile([128, 4, NB], BF16, tag="h6sb")
            for mt in range(4):
                nc.scalar.activation(h6sb[:, mt, :], psum6[mt], RELU, bias=b6t[:, mt:mt + 1])
                nc.sync.dma_start(out=h6_loc[mt * 128:(mt + 1) * 128, :], in_=h6sb[:, mt, :])
            nc.gpsimd.collective_compute(
                "AllGather", mybir.AluOpType.bypass, replica_groups=RG,
                ins=[h6_loc.ap().opt()], outs=[h6_all.ap().opt()])

            psum7 = []
            for _i in range(4):
                p7 = fcps.tile([128, NB], F32, tag="fcps")
                psum7.append(p7)
            for kt in range(32):
                hk = fcap.tile([128, NB], BF16, tag="h6k")
                nc.sync.dma_start(out=hk, in_=h6_all[kt * 128:(kt + 1) * 128, :])
                wt = fcwp.tile([128, 512], BF16, tag="w7")
                nc.sync.dma_start(out=wt, in_=w7t[kt])
                for mt in range(4):
                    nc.tensor.matmul(psum7[mt], wt[:, mt * 128:(mt + 1) * 128],
                                     hk, start=(kt == 0), stop=(kt == 31))
            h7sb = fcop.tile([128, 4, NB], BF16, tag="h7sb")
            for mt in range(4):
                nc.scalar.activation(h7sb[:, mt, :], psum7[mt], RELU, bias=b7t[:, mt:mt + 1])
                nc.sync.dma_start(out=h7_loc[mt * 128:(mt + 1) * 128, :], in_=h7sb[:, mt, :])
            nc.gpsimd.collective_compute(
                "AllGather", mybir.AluOpType.bypass, replica_groups=RG,
                ins=[h7_loc.ap().opt()], outs=[h7_all.ap().opt()])

            psum8 = fcps.tile([125, NB], F32, tag="fcps")
            for kt in range(32):
                hk = fcap.tile([128, NB], BF16, tag="h7k")
                nc.sync.dma_start(out=hk, in_=h7_all[kt * 128:(kt + 1) * 128, :])
                wt = fcwp.tile([128, 125], BF16, tag="w8")
                nc.sync.dma_start(out=wt, in_=w8t[kt])
                nc.tensor.matmul(psum8, wt, hk, start=(kt == 0), stop=(kt == 31))
            o8 = fcop.tile([125, NB], F32, tag="o8")
            nc.scalar.activation(o8, psum8, RELU, bias=b8t[:, 0:1])
            nc.sync.dma_start(out=out8[:, :], in_=o8)

    nc.compile()
    return nc


# ---------------- host-side preprocessing ----------------

def _prep_shared(W1, b1, W2, b2, W3, b3, W4, b4, W5, b5):
    d = {}
    # conv1: pad 11x11 -> 12x12, space-to-depth phases
    W1p = np.zeros((96, 3, 12, 12), np.float32)
    W1p[:, :, :11, :11] = W1
    w1 = W1p.reshape(96, 3, 3, 4, 3, 4).transpose(2, 4, 1, 3, 5, 0)  # dy,dx,c,py,px,co
    d["w1"] = np.ascontiguousarray(w1.reshape(9, 48, 96)).astype(BF)
    d["w2"] = np.ascontiguousarray(
        W2.transpose(2, 3, 1, 0).reshape(25, 96, 256)).astype(BF)
    d["w3"] = np.ascontiguousarray(
        W3.transpose(2, 3, 1, 0).reshape(9, 256, 384)).astype(BF)
    d["w4"] = np.ascontiguousarray(
        W4.transpose(2, 3, 1, 0).reshape(9, 384, 384)).astype(BF)
    d["w5"] = np.ascontiguousarray(
        W5.transpose(2, 3, 1, 0).reshape(9, 384, 256)).astype(BF)
    d["b1s"] = np.ascontiguousarray(b1.reshape(96, 1)).astype(np.float32)
    d["b2s"] = np.ascontiguousarray(b2.reshape(2, 128).T).astype(np.float32)
    d["b3s"] = np.ascontiguousarray(b3.reshape(3, 128).T).astype(np.float32)
    d["b4s"] = np.ascontiguousarray(b4.reshape(3, 128).T).astype(np.float32)
    d["b5s"] = np.ascontiguousarray(b5.reshape(2, 128).T).astype(np.float32)
    idx = np.arange(96)
    d["band96"] = (np.abs(idx[:, None] - idx[None, :]) <= 2).astype(BF)
    idx = np.arange(256)
    a256 = (np.abs(idx[:, None] - idx[None, :]) <= 2).astype(BF)
    d["band256"] = np.ascontiguousarray(
        a256.reshape(2, 128, 2, 128).transpose(1, 0, 2, 3))
    return d


def _prep_x(x):
    B = x.shape[0]
    xp = np.pad(x, ((0, 0), (0, 0), (2, 2), (2, 2)))
    xs = xp.reshape(B, 3, 57, 4, 57, 4).transpose(0, 1, 3, 5, 2, 4)  # B,c,py,px,y,x
    return np.ascontiguousarray(xs.reshape(B, 48, 57, 57)).astype(BF)


def _prep_core(j, W6, b6, W7, b7, W8, b8):
    d = {}
    d["w6t"] = np.ascontiguousarray(
        W6[j * 512:(j + 1) * 512].T.reshape(72, 128, 512)).astype(BF)
    d["w7t"] = np.ascontiguousarray(
        W7[j * 512:(j + 1) * 512].T.reshape(32, 128, 512)).astype(BF)
    d["w8t"] = np.ascontiguousarray(
        W8[j * 125:(j + 1) * 125].T.reshape(32, 128, 125)).astype(BF)
    d["b6s"] = np.ascontiguousarray(
        b6[j * 512:(j + 1) * 512].reshape(4, 128).T).astype(np.float32)
    d["b7s"] = np.ascontiguousarray(
        b7[j * 512:(j + 1) * 512].reshape(4, 128).T).astype(np.float32)
    d["b8s"] = np.ascontiguousarray(
        b8[j * 125:(j + 1) * 125].reshape(125, 1)).astype(np.float32)
    return d


def make_in_maps(inputs, n_img):
    f = {k: np.asarray(v, np.float32) for k, v in inputs.items()}
    shared = _prep_shared(f["W1"], f["b1"], f["W2"], f["b2"], f["W3"], f["b3"],
                          f["W4"], f["b4"], f["W5"], f["b5"])
    xs = _prep_x(f["x"])
    in_maps = []
    for j in range(N_CORES):
        m = dict(shared)
        m.update(_prep_core(j, f["W6"], f["b6"], f["W7"], f["b7"], f["W8"], f["b8"]))
        m["x"] = np.ascontiguousarray(xs[j * n_img:(j + 1) * n_img])
        in_maps.append(m)
    return in_maps


_cached = {}


def kernel(**inputs):
    B = np.asarray(inputs["x"]).shape[0]
    assert B % N_CORES == 0
    n_img = B // N_CORES
    if n_img not in _cached:
        _cached[n_img] = build(n_img)
    nc = _cached[n_img]
    in_maps = make_in_maps(inputs, n_img)
    res = run_bass_kernel_spmd(nc, in_maps, core_ids=list(range(N_CORES)))
    kernel.last_result = res
    out = np.concatenate([res.results[j]["out8"] for j in range(N_CORES)], axis=0)
    return np.ascontiguousarray(out.T.astype(np.float32))


# revision 14
# speedup vs baseline: 1.3884x; 1.3884x over previous
"""AlexNet forward (batch 128) on 8 TRN2 NeuronCores.

Strategy:
- Data-parallel conv stack: 16 images/core. Convs as shift-matmuls in bf16
  (fp32 PSUM accum). conv1's 11x11/s4 conv is rewritten via space-to-depth
  (4x4 phases) into a 3x3/s1 conv over 48 channels.
- LRN via band-matrix matmul for the channel-window sum of squares, then a
  first-order expansion (k + a*S)^-b ~= C0 - C1*S (exact to ~1e-8 here since
  a*S << k).
- Maxpool 3x3/s2 as strided DVE max ops.
- FC layers tensor-sharded 8 ways (512/512/125 output columns per core) over
  the full batch-128, with AllGather collectives between layers.
Host side: pad/space-to-depth/transpose/cast of inputs+weights, final
concat+transpose of the per-core FC8 slices.
"""
import numpy as np
import ml_dtypes

import concourse.bass as bass
import concourse.mybir as mybir
import concourse.tile as tile
from concourse import bacc
from concourse.bass_utils import run_bass_kernel_spmd

F32 = mybir.dt.float32
BF16 = mybir.dt.float16
BF = np.float16

N_CORES = 8
C0 = float(2.0 ** -0.75)
C1 = float(0.75 * 1e-4 * 2.0 ** -1.75)
RELU = mybir.ActivationFunctionType.Relu
COPY = mybir.ActivationFunctionType.Copy
MAX = mybir.AluOpType.max
MULT = mybir.AluOpType.mult


def _chunks(total, maxn):
    n = -(-total // maxn)
    base, rem = divmod(total, n)
    out, s = [], 0
    for i in range(n):
        ln = base + (1 if i < rem else 0)
        out.append((s, ln))
        s += ln
    return out


def _groups(n, g):
    out, s = [], 0
    while s < n:
        out.append((s, min(g, n - s)))
        s += min(g, n - s)
    return out


def _pool_cols(v, wo):
    """3-wide stride-2 max along the last dim of a [P, H, Wi] view -> [P, H, wo]."""
    return [v[:, :, s: s + 2 * (wo - 1) + 1: 2] for s in range(3)]


def _pool_rows(v, ho):
    return [v[:, s: s + 2 * (ho - 1) + 1: 2, :] for s in range(3)]


def build(n_img=16):
    NB = N_CORES * n_img
    nc = bacc.Bacc(num_devices=N_CORES)

    def inp(name, shape, dt=BF16):
        return nc.declare_dram_parameter(name, list(shape), dt, isOutput=False)

    x = inp("x", (n_img, 48, 57, 57))
    w1 = inp("w1", (9, 48, 96))
    w2 = inp("w2", (5, 480, 256))
    w3 = inp("w3", (9, 256, 384))
    w4 = inp("w4", (9, 384, 384))
    w5 = inp("w5", (9, 384, 256))
    w6t = inp("w6t", (72, 128, 512))
    w7t = inp("w7t", (32, 128, 512))
    w8t = inp("w8t", (32, 128, 125))
    b1s = inp("b1s", (96, 1), F32)
    b2s = inp("b2s", (128, 2), F32)
    b3s = inp("b3s", (128, 3), F32)
    b4s = inp("b4s", (128, 3), F32)
    b5s = inp("b5s", (128, 2), F32)
    b6r = inp("b6r", (1, 512))
    b7r = inp("b7r", (1, 512))
    b8r = inp("b8r", (1, 125))
    identd = inp("ident", (128, 128))
    band96d = inp("band96", (96, 96))
    band256d = inp("band256", (128, 2, 2, 128))

    out8 = nc.declare_dram_parameter("out8", [NB, 125], F32, isOutput=True)

    h5_loc = nc.dram_tensor("h5_loc", [n_img, 2, 128, 36], BF16)
    h5_all = nc.dram_tensor("h5_all", [NB, 9216], BF16, addr_space="Shared")
    h6_loc = nc.dram_tensor("h6_loc", [512, NB], BF16)
    h6_all = nc.dram_tensor("h6_all", [4096, NB], BF16, addr_space="Shared")
    h7_loc = nc.dram_tensor("h7_loc", [512, NB], BF16)
    h7_all = nc.dram_tensor("h7_all", [4096, NB], BF16, addr_space="Shared")

    RG = [list(range(N_CORES))]

    with tile.TileContext(nc, num_cores=N_CORES) as tc:
        with (
            tc.tile_pool(name="consts", bufs=1) as consts,
            tc.tile_pool(name="convw", bufs=1) as convw,
            tc.tile_pool(name="xin", bufs=2) as xinp,
            tc.tile_pool(name="act", bufs=2) as actp,
            tc.tile_pool(name="chunk", bufs=2) as chp,
            tc.tile_pool(name="grp", bufs=2) as grpp,
            tc.tile_pool(name="fcw", bufs=8) as fcwp,
            tc.tile_pool(name="fca", bufs=4) as fcap,
            tc.tile_pool(name="fco", bufs=1) as fcop,
            tc.tile_pool(name="convps", bufs=4, space="PSUM") as convps,
            tc.tile_pool(name="lrnps", bufs=2, space="PSUM") as lrnps,
            tc.tile_pool(name="fcps", bufs=2, space="PSUM") as fcps,
        ):
            # ---------------- constants / conv weights ----------------
            b1t = consts.tile([96, 1], F32); nc.sync.dma_start(out=b1t, in_=b1s[:, :])
            b2t = consts.tile([128, 2], F32); nc.sync.dma_start(out=b2t, in_=b2s[:, :])
            b3t = consts.tile([128, 3], F32); nc.sync.dma_start(out=b3t, in_=b3s[:, :])
            b4t = consts.tile([128, 3], F32); nc.sync.dma_start(out=b4t, in_=b4s[:, :])
            b5t = consts.tile([128, 2], F32); nc.sync.dma_start(out=b5t, in_=b5s[:, :])
            b6rt = consts.tile([1, 512], BF16); nc.sync.dma_start(out=b6rt, in_=b6r[:, :])
            b7rt = consts.tile([1, 512], BF16); nc.sync.dma_start(out=b7rt, in_=b7r[:, :])
            b8rt = consts.tile([1, 125], BF16); nc.sync.dma_start(out=b8rt, in_=b8r[:, :])
            ones = consts.tile([1, 128], BF16); nc.vector.memset(ones, 1.0)
            ident = consts.tile([128, 128], BF16)
            nc.sync.dma_start(out=ident, in_=identd[:, :])
            band96 = consts.tile([96, 96], BF16)
            nc.sync.dma_start(out=band96, in_=band96d[:, :])
            band256 = consts.tile([128, 2, 2, 128], BF16)

            w1t = convw.tile([112, 9, 96], BF16)
            nc.sync.dma_start(out=w1t[0:48], in_=w1.ap().rearrange("p c o -> c p o"))
            nc.sync.dma_start(out=w1t[64:112], in_=w1.ap().rearrange("p c o -> c p o"))
            w2ts = []
            for _t in range(4):
                w2tt = convw.tile([120, 5, 256], BF16, tag=f"w2t{_t}")
                w2ts.append(w2tt)
            w3t = convw.tile([128, 2, 9, 384], BF16, tag="w3t")
            w4t = convw.tile([128, 3, 9, 384], BF16, tag="w4t")
            w5t = convw.tile([128, 3, 9, 256], BF16, tag="w5t")
            CH55 = _chunks(55, 9)   # conv1/lrn1 row chunks (N<=495)
            CH27 = _chunks(27, 18)  # conv2 row chunks (N<=486)

            # ---------------- conv stack, software-pipelined over images ----------
            # PE emission order: A(0), then per image i: L(i), A(i+1), B(i), M(i),
            # and at group ends G(g) [conv3/4/5+pool5]. A(i+1) fills the PE gap
            # that pool1(i) would otherwise create (HAM stays warm).
            st = {}
            GRPS = _groups(n_img, 3)

            def stage_A(i):  # conv1 + relu -> r1 (chunk pairs packed in PE row groups)
                xin = xinp.tile([112, 57, 57], BF16, tag="xin")
                nc.sync.dma_start(out=xin[0:48], in_=x[i])
                nc.sync.dma_start(out=xin[64:112], in_=x[i])
                r1 = actp.tile([96, 55, 55], BF16, tag="r1")
                pairs = [(CH55[c], CH55[c + 1] if c + 1 < len(CH55) else None)
                         for c in range(0, len(CH55), 2)]
                for (ca, cb) in pairs:
                    psa = convps.tile([96, ca[1], 55], F32, tag="cps")
                    psb = None
                    if cb is not None:
                        psb = convps.tile([96, cb[1], 55], F32, tag="cps")
                    k = 0
                    for dy in range(3):
                        for dx in range(3):
                            nc.tensor.matmul(
                                psa, w1t[0:48, dy * 3 + dx, :],
                                xin[0:48, dy + ca[0]: dy + ca[0] + ca[1], dx: dx + 55],
                                start=(k == 0), stop=(k == 8), tile_position=(0, 0))
                            if cb is not None:
                                nc.tensor.matmul(
                                    psb, w1t[64:112, dy * 3 + dx, :],
                                    xin[64:112, dy + cb[0]: dy + cb[0] + cb[1], dx: dx + 55],
                                    start=(k == 0), stop=(k == 8), tile_position=(64, 0))
                            k += 1
                    nc.scalar.activation(r1[:, ca[0]:ca[0] + ca[1], :], psa, RELU, bias=b1t[:, 0:1])
                    if cb is not None:
                        nc.scalar.activation(r1[:, cb[0]:cb[0] + cb[1], :], psb, RELU, bias=b1t[:, 0:1])
                st[i] = {"r1": r1}

            def stage_L(i):  # lrn1 (in place on r1) + pool1 -> in2
                r1 = st[i]["r1"]
                for (r0, nr) in CH55:
                    rc = r1[:, r0:r0 + nr, :]
                    t1 = chp.tile([96, nr, 55], BF16, tag="t1")
                    nc.vector.tensor_mul(t1, rc, rc)
                    sp = lrnps.tile([96, nr, 55], F32, tag="lps")
                    nc.tensor.matmul(sp, band96, t1, start=True, stop=True)
                    wc = chp.tile([96, nr, 55], F32, tag="wc")
                    nc.scalar.activation(wc, sp, COPY, bias=C0, scale=-C1)
                    nc.vector.tensor_tensor(out=rc, in0=rc, in1=wc, op=MULT)
                in2 = actp.tile([96, 965], BF16, tag="in2")
                nc.vector.memset(in2[:, :], 0.0)
                in2v = in2[:, 0:961].rearrange("c (a b) -> c a b", a=31)
                tmp1 = chp.tile([96, 55, 27], BF16, tag="tmp1")
                ca, cb, cc = _pool_cols(r1, 27)
                nc.vector.tensor_tensor(out=tmp1, in0=ca, in1=cb, op=MAX)
                nc.vector.tensor_tensor(out=tmp1, in0=tmp1, in1=cc, op=MAX)
                ra, rb, rw = _pool_rows(tmp1, 27)
                p1v = in2v[:, 2:29, 2:29]
                nc.vector.tensor_tensor(out=p1v, in0=ra, in1=rb, op=MAX)
                nc.vector.tensor_tensor(out=p1v, in0=p1v, in1=rw, op=MAX)
                st[i]["in2"] = in2

            def stage_B(i):  # conv2 + relu -> r2, (ci,kx)-packed K=480 over 4x120
                in2 = st[i]["in2"]
                xd = []
                for _t in range(4):
                    xdt = chp.tile([120, 961], BF16, tag=f"xd{_t}")
                    src = bass.AP(
                        tensor=in2.tensor, offset=in2.offset,
                        ap=[[in2.ap[0][0], 0], [1, 0], [1, 961]])
                    # partition (ci), kx, n dims: build via explicit AP
                    src.ap[0] = [in2.ap[0][0], 24]
                    src.ap[1] = [1, 5]
                    src.offset = in2.offset + _t * 24 * in2.ap[0][0]
                    nc.sync.dma_start(out=xdt, in_=src)
                    xd.append(xdt)
                r2 = actp.tile([128, 2, 27, 27], BF16, tag="r2")
                for (r0, nr) in CH27:
                    for mt in range(2):
                        ps = convps.tile([128, nr, 27], F32, tag="cps")
                        k = 0
                        for ky in range(5):
                            for _t in range(4):
                                rhs = bass.AP(
                                    tensor=xd[_t].tensor, offset=xd[_t].offset + ky * 31 + r0 * 31,
                                    ap=[list(xd[_t].ap[0]), [31, nr], [1, 27]])
                                nc.tensor.matmul(
                                    ps, w2ts[_t][:, ky, mt * 128:(mt + 1) * 128], rhs,
                                    start=(k == 0), stop=(k == 19))
                                k += 1
                        nc.scalar.activation(r2[:, mt, r0:r0 + nr, :], ps, RELU,
                                             bias=b2t[:, mt:mt + 1])
                st[i]["r2"] = r2

            def stage_M(i, sl, in3):  # lrn2 (in place) + pool2 -> in3 slot
                r2 = st[i]["r2"]
                for (r0, nr) in CH27:
                    t2 = chp.tile([128, 2, nr, 27], BF16, tag="t2")
                    for kt in range(2):
                        rc = r2[:, kt, r0:r0 + nr, :]
                        nc.vector.tensor_mul(t2[:, kt, :, :], rc, rc)
                    for mt in range(2):
                        sp = lrnps.tile([128, nr, 27], F32, tag="lps")
                        for kt in range(2):
                            nc.tensor.matmul(sp, band256[:, kt, mt, :], t2[:, kt, :, :],
                                             start=(kt == 0), stop=(kt == 1))
                        wc = chp.tile([128, nr, 27], F32, tag="wc2")
                        nc.scalar.activation(wc, sp, COPY, bias=C0, scale=-C1)
                        rc = r2[:, mt, r0:r0 + nr, :]
                        nc.vector.tensor_tensor(out=rc, in0=rc, in1=wc, op=MULT)
                for kt in range(2):
                    tmp2 = chp.tile([128, 27, 13], BF16, tag="tmp2")
                    ca, cb, cc = _pool_cols(r2[:, kt, :, :], 13)
                    nc.vector.tensor_tensor(out=tmp2, in0=ca, in1=cb, op=MAX)
                    nc.vector.tensor_tensor(out=tmp2, in0=tmp2, in1=cc, op=MAX)
                    ra, rb, rw = _pool_rows(tmp2, 13)
                    pv = in3[:, kt, sl, 1:14, 1:14]
                    nc.vector.tensor_tensor(out=pv, in0=ra, in1=rb, op=MAX)
                    nc.vector.tensor_tensor(out=pv, in0=pv, in1=rw, op=MAX)
                del st[i]["r1"], st[i]["in2"]

            def conv345(w, in_t, nkt, out_v, bias_t, mt_range, glen):
                for mt in mt_range:
                    ps = convps.tile([128, glen, 13, 13], F32, tag="cps")
                    k, klast = 0, nkt * 9 - 1
                    for kt in range(nkt):
                        for dy in range(3):
                            for dx in range(3):
                                nc.tensor.matmul(
                                    ps, w[:, kt, dy * 3 + dx, mt * 128:(mt + 1) * 128],
                                    in_t[:, kt, 0:glen, dy:dy + 13, dx:dx + 13],
                                    start=(k == 0), stop=(k == klast))
                                k += 1
                    yield mt, ps

            def stage_G(g0, glen, in3):  # conv3/4/5 + pool5 + h5 dma for group
                in4 = grpp.tile([128, 3, 3, 15, 15], BF16, tag="in4")
                nc.vector.memset(in4[:, :, :, :, :], 0.0)
                for mt, ps in conv345(w3t, in3, 2, in4, b3t, range(3), glen):
                    nc.scalar.activation(in4[:, mt, 0:glen, 1:14, 1:14], ps, RELU,
                                         bias=b3t[:, mt:mt + 1])
                in5 = grpp.tile([128, 3, 3, 15, 15], BF16, tag="in5")
                nc.vector.memset(in5[:, :, :, :, :], 0.0)
                for mt, ps in conv345(w4t, in4, 3, in5, b4t, range(3), glen):
                    nc.scalar.activation(in5[:, mt, 0:glen, 1:14, 1:14], ps, RELU,
                                         bias=b4t[:, mt:mt + 1])
                r5 = grpp.tile([128, 2, 3, 13, 13], BF16, tag="r5")
                for mt, ps in conv345(w5t, in5, 3, r5, b5t, range(2), glen):
                    nc.scalar.activation(r5[:, mt, 0:glen, :, :], ps, RELU,
                                         bias=b5t[:, mt:mt + 1])
                for i in range(g0, g0 + glen):
                    sl = i - g0
                    for kt in range(2):
                        tmp5 = chp.tile([128, 13, 6], BF16, tag="tmp5")
                        ca, cb, cc = _pool_cols(r5[:, kt, sl, :, :], 6)
                        nc.vector.tensor_tensor(out=tmp5, in0=ca, in1=cb, op=MAX)
                        nc.vector.tensor_tensor(out=tmp5, in0=tmp5, in1=cc, op=MAX)
                        h5sb = chp.tile([128, 6, 6], BF16, tag="h5sb")
                        ra, rb, rw = _pool_rows(tmp5, 6)
                        nc.vector.tensor_tensor(out=h5sb, in0=ra, in1=rb, op=MAX)
                        nc.vector.tensor_tensor(out=h5sb, in0=h5sb, in1=rw, op=MAX)
                        nc.sync.dma_start(out=h5_loc[i, kt], in_=h5sb)

            N_AG_CHUNKS = 4 if n_img % 4 == 0 else 1
            AG_CH = n_img // N_AG_CHUNKS
            ag_done = []
            in3_tiles = {}
            stage_A(0)
            for _t in range(4):
                nc.sync.dma_start(out=w2ts[_t],
                                  in_=w2[:, _t * 120:(_t + 1) * 120, :].rearrange("p c o -> c p o"))
            nc.sync.dma_start(out=band256, in_=band256d[:, :, :, :])
            for kt in range(2):
                nc.sync.dma_start(
                    out=w3t[:, kt, :, :],
                    in_=w3[:, kt * 128:(kt + 1) * 128, :].rearrange("p c o -> c p o"))
            for kt in range(3):
                nc.sync.dma_start(
                    out=w4t[:, kt, :, :],
                    in_=w4[:, kt * 128:(kt + 1) * 128, :].rearrange("p c o -> c p o"))
            for kt in range(3):
                nc.sync.dma_start(
                    out=w5t[:, kt, :, :],
                    in_=w5[:, kt * 128:(kt + 1) * 128, :].rearrange("p c o -> c p o"))
            for i in range(n_img):
                g = i // 3
                g0, glen = GRPS[g]
                sl = i - g0
                if sl == 0:
                    in3g = grpp.tile([128, 2, 3, 15, 15], BF16, tag="in3")
                    nc.vector.memset(in3g[:, :, :, :, :], 0.0)
                    in3_tiles[g] = in3g
                stage_L(i)
                if i + 1 < n_img:
                    stage_A(i + 1)
                stage_B(i)
                stage_M(i, sl, in3_tiles[g])
                if sl == glen - 1:
                    stage_G(g0, glen, in3_tiles[g])
                    del in3_tiles[g]
                    while (len(ag_done) < N_AG_CHUNKS
                           and (len(ag_done) + 1) * AG_CH - 1 <= g0 + glen - 1):
                        k = len(ag_done)
                        nc.gpsimd.collective_compute(
                            "AllGather", mybir.AluOpType.bypass, replica_groups=RG,
                            ins=[h5_loc[k * AG_CH:(k + 1) * AG_CH].opt()],
                            outs=[h5_all[k * AG_CH * N_CORES:(k + 1) * AG_CH * N_CORES, :].opt()])
                        ag_done.append(k)

            # ---------------- FC phase ----------------

            # fc6 (batch-major): z6[b, o] = sum_k h5T[k, b]^T @ w6t[k, o] + b6
            hall = fcop.tile([NB, 72, 128], BF16, tag="hall")
            nc.sync.dma_start(out=hall, in_=h5_all[:, :])
            z6 = fcps.tile([NB, 512], F32, tag="fcps")
            for kt in range(72):
                tp = lrnps.tile([128, NB], BF16, tag="lps")
                nc.tensor.transpose(tp, hall[:, kt, :], ident[:NB, :NB])
                hk = fcap.tile([128, NB], BF16, tag="h5k")
                nc.scalar.activation(hk, tp, COPY)
                wt = fcwp.tile([128, 512], BF16, tag="w6")
                nc.sync.dma_start(out=wt, in_=w6t[kt])
                nc.tensor.matmul(z6, hk, wt, start=(kt == 0), stop=False)
            nc.tensor.matmul(z6, ones[:, :NB], b6rt, start=False, stop=True)
            z6s = fcop.tile([NB, 512], BF16, tag="z6s")
            nc.scalar.activation(z6s, z6, RELU)
            h6sb = fcop.tile([128, 4, NB], BF16, tag="h6sb")
            for mt in range(4):
                tp = lrnps.tile([128, NB], BF16, tag="lps")
                nc.tensor.transpose(tp, z6s[:, mt * 128:(mt + 1) * 128], ident[:NB, :NB])
                nc.scalar.activation(h6sb[:, mt, :], tp, COPY)
                nc.sync.dma_start(out=h6_loc[mt * 128:(mt + 1) * 128, :], in_=h6sb[:, mt, :])
            nc.gpsimd.collective_compute(
                "AllGather", mybir.AluOpType.bypass, replica_groups=RG,
                ins=[h6_loc.ap().opt()], outs=[h6_all.ap().opt()])

            z7 = fcps.tile([NB, 512], F32, tag="fcps")
            for kt in range(32):
                hk = fcap.tile([128, NB], BF16, tag="h6k")
                nc.sync.dma_start(out=hk, in_=h6_all[kt * 128:(kt + 1) * 128, :])
                wt = fcwp.tile([128, 512], BF16, tag="w7")
                nc.sync.dma_start(out=wt, in_=w7t[kt])
                nc.tensor.matmul(z7, hk, wt, start=(kt == 0), stop=False)
            nc.tensor.matmul(z7, ones[:, :NB], b7rt, start=False, stop=True)
            z7s = fcop.tile([NB, 512], BF16, tag="z7s")
            nc.scalar.activation(z7s, z7, RELU)
            h7sb = fcop.tile([128, 4, NB], BF16, tag="h7sb")
            for mt in range(4):
                tp = lrnps.tile([128, NB], BF16, tag="lps")
                nc.tensor.transpose(tp, z7s[:, mt * 128:(mt + 1) * 128], ident[:NB, :NB])
                nc.scalar.activation(h7sb[:, mt, :], tp, COPY)
                nc.sync.dma_start(out=h7_loc[mt * 128:(mt + 1) * 128, :], in_=h7sb[:, mt, :])
            nc.gpsimd.collective_compute(
                "AllGather", mybir.AluOpType.bypass, replica_groups=RG,
                ins=[h7_loc.ap().opt()], outs=[h7_all.ap().opt()])

            z8 = fcps.tile([NB, 125], F32, tag="fcps")
            for kt in range(32):
                hk = fcap.tile([128, NB], BF16, tag="h7k")
                nc.sync.dma_start(out=hk, in_=h7_all[kt * 128:(kt + 1) * 128, :])
                wt = fcwp.tile([128, 125], BF16, tag="w8")
                nc.sync.dma_start(out=wt, in_=w8t[kt])
                nc.tensor.matmul(z8, hk, wt, start=(kt == 0), stop=False)
            nc.tensor.matmul(z8, ones[:, :NB], b8rt, start=False, stop=True)
            o8 = fcop.tile([NB, 125], F32, tag="o8")
            nc.scalar.activation(o8, z8, RELU)
            nc.sync.dma_start(out=out8[:, :], in_=o8)

    nc.compile()
    return nc


# ---------------- host-side preprocessing ----------------

def _prep_shared(W1, b1, W2, b2, W3, b3, W4, b4, W5, b5):
    d = {}
    # conv1: pad 11x11 -> 12x12, space-to-depth phases
    W1p = np.zeros((96, 3, 12, 12), np.float32)
    W1p[:, :, :11, :11] = W1
    w1 = W1p.reshape(96, 3, 3, 4, 3, 4).transpose(2, 4, 1, 3, 5, 0)  # dy,dx,c,py,px,co
    d["w1"] = np.ascontiguousarray(w1.reshape(9, 48, 96)).astype(BF)
    d["w2"] = np.ascontiguousarray(
        W2.transpose(2, 1, 3, 0).reshape(5, 480, 256)).astype(BF)
    d["w3"] = np.ascontiguousarray(
        W3.transpose(2, 3, 1, 0).reshape(9, 256, 384)).astype(BF)
    d["w4"] = np.ascontiguousarray(
        W4.transpose(2, 3, 1, 0).reshape(9, 384, 384)).astype(BF)
    d["w5"] = np.ascontiguousarray(
        W5.transpose(2, 3, 1, 0).reshape(9, 384, 256)).astype(BF)
    d["b1s"] = np.ascontiguousarray(b1.reshape(96, 1)).astype(np.float32)
    d["b2s"] = np.ascontiguousarray(b2.reshape(2, 128).T).astype(np.float32)
    d["b3s"] = np.ascontiguousarray(b3.reshape(3, 128).T).astype(np.float32)
    d["b4s"] = np.ascontiguousarray(b4.reshape(3, 128).T).astype(np.float32)
    d["b5s"] = np.ascontiguousarray(b5.reshape(2, 128).T).astype(np.float32)
    d["ident"] = np.eye(128, dtype=BF)
    idx = np.arange(96)
    d["band96"] = (np.abs(idx[:, None] - idx[None, :]) <= 2).astype(BF)
    idx = np.arange(256)
    a256 = (np.abs(idx[:, None] - idx[None, :]) <= 2).astype(BF)
    d["band256"] = np.ascontiguousarray(
        a256.reshape(2, 128, 2, 128).transpose(1, 0, 2, 3))
    return d


def _prep_x(x):
    B = x.shape[0]
    xp = np.pad(x, ((0, 0), (0, 0), (2, 2), (2, 2)))
    xs = xp.reshape(B, 3, 57, 4, 57, 4).transpose(0, 1, 3, 5, 2, 4)  # B,c,py,px,y,x
    return np.ascontiguousarray(xs.reshape(B, 48, 57, 57)).astype(BF)


def _prep_core(j, W6, b6, W7, b7, W8, b8):
    d = {}
    d["w6t"] = np.ascontiguousarray(
        W6[j * 512:(j + 1) * 512].T.reshape(72, 128, 512)).astype(BF)
    d["w7t"] = np.ascontiguousarray(
        W7[j * 512:(j + 1) * 512].T.reshape(32, 128, 512)).astype(BF)
    d["w8t"] = np.ascontiguousarray(
        W8[j * 125:(j + 1) * 125].T.reshape(32, 128, 125)).astype(BF)
    d["b6r"] = np.ascontiguousarray(b6[j * 512:(j + 1) * 512].reshape(1, 512)).astype(BF)
    d["b7r"] = np.ascontiguousarray(b7[j * 512:(j + 1) * 512].reshape(1, 512)).astype(BF)
    d["b8r"] = np.ascontiguousarray(b8[j * 125:(j + 1) * 125].reshape(1, 125)).astype(BF)
    return d


def make_in_maps(inputs, n_img):
    f = {k: np.asarray(v, np.float32) for k, v in inputs.items()}
    shared = _prep_shared(f["W1"], f["b1"], f["W2"], f["b2"], f["W3"], f["b3"],
                          f["W4"], f["b4"], f["W5"], f["b5"])
    xs = _prep_x(f["x"])
    in_maps = []
    for j in range(N_CORES):
        m = dict(shared)
        m.update(_prep_core(j, f["W6"], f["b6"], f["W7"], f["b7"], f["W8"], f["b8"]))
        C = 4 if n_img % 4 == 0 else 1
        sz = n_img // C
        idxs = [N_CORES * sz * k + sz * j + c for k in range(C) for c in range(sz)]
        m["x"] = np.ascontiguousarray(xs[idxs])
        in_maps.append(m)
    return in_maps


_cached = {}


def kernel(**inputs):
    B = np.asarray(inputs["x"]).shape[0]
    assert B % N_CORES == 0
    n_img = B // N_CORES
    if n_img not in _cached:
        _cached[n_img] = build(n_img)
    nc = _cached[n_img]
    in_maps = make_in_maps(inputs, n_img)
    res = run_bass_kernel_spmd(nc, in_maps, core_ids=list(range(N_CORES)))
    kernel.last_result = res
    out = np.concatenate([res.results[j]["out8"] for j in range(N_CORES)], axis=1)
    return np.ascontiguousarray(out.astype(np.float32))
